# revision 13
# baseline (speedup 1.0000x reference)
"""Trainium2 Bass kernel for a dense transformer block (B=4, T=2048, C=1024,
H=16, FF=4096, causal attention, fp32 I/O).

Sharding: data-parallel over 8 cores, 2 cores per batch, zigzag 128-row query
chunks (ZIG) to balance causal attention across the pair under one SPMD
program. K/V recomputed per core for the full batch.

Precision: fp8e4m3 DoubleRow matmuls everywhere except LN/softmax/residual
arithmetic. Error control:
- power-of-2 scales on every fp8 tensor, folded into existing epilogue
  scalars / the exp bias (zero extra ops);
- MLP runs 3-term split products (W_hi z_hi + W_lo z_hi + W_hi z_lo) so both
  weight- and activation-quantization errors cancel to O(fp8^2);
- attention keeps plain fp8 (softmax averaging suppresses the noise).
z^T layouts for Q and the MLP are produced by DMA-xbar transposes of fp8
pairs viewed as uint16, which lands tensors directly in DoubleRow pair
layout with no PE/ACT/DVE cost.
"""

import sys

for _p in ("/opt/trn_rl_repo",):
    if _p not in sys.path:
        sys.path.insert(0, _p)

import numpy as np
import ml_dtypes

import concourse.bass as bass
import concourse.mybir as mybir
import concourse.tile as tile
from concourse import bacc
from concourse.bass_utils import run_bass_kernel_spmd
from concourse.masks import make_identity

BF16 = ml_dtypes.bfloat16
FP8 = ml_dtypes.float8_e4m3fn
F32 = mybir.dt.float32
BF = mybir.dt.bfloat16
F8 = mybir.dt.float8e4
U16 = mybir.dt.uint16

EMB = 1024
HEADS = 16
HD = 64
FF = 4096
T = 2048
B = 4
EPS = 1e-5
TQ = 1024  # own query rows per core
NJ = 8  # own 128-row chunks per core
NS = 16  # key slots (128 keys each)
ZIG = [[0, 3, 4, 7, 8, 11, 12, 15], [1, 2, 5, 6, 9, 10, 13, 14]]

# physical fp8 scales (compile-time)
SZ = 8.0     # z1/z2
SQ = 4.0     # qt/kt
SP = 8.0     # p = exp
SV = 32.0    # v
SO = 32.0    # attention out (oT)
SU = 16.0    # u
EXP_BIAS = float(-5.0 + np.log(SP))  # exp(score/SQ^2 + EXP_BIAS)
S_WQ = 2048.0
S_WK = 2048.0
S_WV = 1024.0
S_WO = 1024.0
S_W1 = 1024.0
S_W2 = 2048.0

# packed P^T column offsets: slot s covers own-chunk range [s//2, 8)
PT_OFF = [0] * NS
for _s in range(1, NS):
    PT_OFF[_s] = PT_OFF[_s - 1] + (NJ - (_s - 1) // 2) * 128
PT_W = PT_OFF[-1] + (NJ - (NS - 1) // 2) * 128  # 9216

DR = mybir.MatmulPerfMode.DoubleRow


def _bank_spans(m):
    """fp32 PSUM bank-aligned column spans covering [m*128, 1024)."""
    if m < 4:
        return [(m * 128, 512), (512, 1024)]
    return [(m * 128, 1024)]


def _ln(nc, pools, xt, n_free, eps_t):
    """LayerNorm stats for xt [128, n_free] fp32 -> (mu, rstd*SZ) [128,1]."""
    stats = pools["stats"].tile([128, 2, 6], F32)
    half = n_free // 2
    nc.vector.bn_stats(out=stats[:, 0, :], in_=xt[:, 0:half])
    nc.vector.bn_stats(out=stats[:, 1, :], in_=xt[:, half:n_free])
    mv = pools["stats"].tile([128, 2], F32)
    nc.vector.bn_aggr(out=mv, in_=stats)
    rstd = pools["stats"].tile([128, 1], F32)
    nc.scalar.activation(
        out=rstd, in_=mv[:, 1:2], func=mybir.ActivationFunctionType.Sqrt,
        bias=eps_t, scale=1.0 / (SZ * SZ),
    )
    nc.vector.reciprocal(out=rstd, in_=rstd)
    return mv[:, 0:1], rstd


def _u16t(nc, eng, dst_ap, src_ap):
    """DMA-xbar transpose of an fp8 tile viewed as uint16 pairs.
    src [128, 2W] fp8 row-major -> dst [128, W, 2] fp8: dst[p, t, i] =
    src[t, 2p+i]."""
    eng.dma_start_transpose(
        out=dst_ap.rearrange("p a b -> p (a b)").bitcast(U16),
        in_=src_ap.bitcast(U16))


PHASE_MARKS = []


def build_program():
    from contextlib import ExitStack

    nc = bacc.Bacc("TRN2", target_bir_lowering=False, debug=False, num_devices=1)

    d_xq = nc.dram_tensor("x_q", [TQ, EMB], F32, kind="ExternalInput").ap()
    d_xqb = nc.dram_tensor("x_qb", [TQ, EMB], F32, kind="ExternalInput").ap()
    d_xkv = nc.dram_tensor("x_kv", [T, EMB], F32, kind="ExternalInput").ap()
    # weights host-swizzled (fp8 + scales); wq/w1 in row-pair layouts
    d_wq = nc.dram_tensor("wq", [128, 4, 2, EMB], F8, kind="ExternalInput").ap()
    d_wk = nc.dram_tensor("wk", [128, 8, EMB], F8, kind="ExternalInput").ap()
    d_wv = nc.dram_tensor("wv", [128, 8, EMB], F8, kind="ExternalInput").ap()
    d_wo = nc.dram_tensor("wo", [128, 2, 8, EMB], F8, kind="ExternalInput").ap()
    d_w1 = nc.dram_tensor(
        "w1", [32, 128, 2, 4, 2, 128], F8, kind="ExternalInput").ap()
    d_w2 = nc.dram_tensor(
        "w2", [2, 128, 2, 32, 512], F8, kind="ExternalInput").ap()
    d_bq = nc.dram_tensor("bq", [128, 8], F32, kind="ExternalInput").ap()
    d_bk = nc.dram_tensor("bk", [128, 8], F32, kind="ExternalInput").ap()
    d_b1 = nc.dram_tensor("b1s", [128, 32], F32, kind="ExternalInput").ap()
    d_bv = nc.dram_tensor("bvrow", [1, EMB], F32, kind="ExternalInput").ap()
    d_mm = nc.dram_tensor("maskm", [128, NS, 128], F8, kind="ExternalInput").ap()
    d_y = nc.dram_tensor("y", [TQ, EMB], F32, kind="ExternalOutput").ap()

    Exp = mybir.ActivationFunctionType.Exp
    Relu = mybir.ActivationFunctionType.Relu
    MUL = mybir.AluOpType.mult
    ADD = mybir.AluOpType.add
    SUB = mybir.AluOpType.subtract
    MAX = mybir.AluOpType.max

    with tile.TileContext(nc) as tc, ExitStack() as top:
        consts = top.enter_context(tc.tile_pool(name="consts", bufs=1))
        ident = consts.tile([128, 128], BF)
        make_identity(nc, ident)
        eps_t = consts.tile([128, 1], F32)
        nc.vector.memset(eps_t, EPS)
        expb_t = consts.tile([128, 1], F32)
        nc.vector.memset(expb_t, EXP_BIAS)
        bq_sb = consts.tile([128, 8], F32)
        nc.sync.dma_start(out=bq_sb, in_=d_bq)
        bk_sb = consts.tile([128, 8], F32)
        nc.sync.dma_start(out=bk_sb, in_=d_bk)
        b1_sb = consts.tile([128, 32], F32)
        nc.sync.dma_start(out=b1_sb, in_=d_b1)
        stM = ExitStack()
        mm_sb = stM.enter_context(tc.tile_pool(name="maskp", bufs=1)).tile(
            [128, NS, 128], F8, name="mm_sb")
        nc.sync.dma_start(out=mm_sb, in_=d_mm)

        def bcast_row(dst, src_row):
            b_ap = bass.AP(
                tensor=src_row.tensor, offset=src_row.offset,
                ap=[[0, 128]] + list(src_row.ap[1:]))
            nc.gpsimd.dma_start(out=dst, in_=b_ap)

        bv_sb = consts.tile([128, EMB], F32)
        bcast_row(bv_sb, d_bv)

        pools = {}

        stZ = ExitStack()   # z^T tensors: die after QKV+attn
        stA = ExitStack()   # v: dies after attention
        stO = ExitStack()   # oT_all: dies after Wo
        stX = ExitStack()   # x2/z2T/uT: die at end
        top.enter_context(stX)
        top.enter_context(stO)
        top.enter_context(stA)
        top.enter_context(stZ)

        # zq in u16-transposed pair layout [128, chunk, tok, 2]
        zq_p = stZ.enter_context(tc.tile_pool(name="zqT", bufs=1))
        zkv_p = stZ.enter_context(tc.tile_pool(name="zkvT", bufs=1))
        zqT = zq_p.tile([128, 4, TQ, 2], F8, name="zqT")
        zkc = [zkv_p.tile([128, 8, 512], F8, name=f"zkc{i}") for i in range(4)]

        v_sb = stA.enter_context(
            tc.tile_pool(name="v", bufs=1, side="right")).tile(
            [128, NS, HEADS, 96], F8, name="v_t")
        nc.vector.memset(v_sb[:, :, :, 64:96], 0.0)
        nc.vector.memset(v_sb[:, :, :, 64:65], SV / SO)

        oT_all = stO.enter_context(tc.tile_pool(name="oT", bufs=1)).tile(
            [128, 8, TQ], F8, name="oT_t")
        ph2 = ExitStack()
        ph3 = ExitStack()
        wqk_p = ph3.enter_context(tc.tile_pool(name="wqk", bufs=3))
        qt_p = ph3.enter_context(tc.tile_pool(name="qTot", bufs=2))
        kt_p = ph3.enter_context(tc.tile_pool(name="kTot", bufs=2))
        pt_p = ph3.enter_context(tc.tile_pool(name="pT", bufs=2))
        rd_p = ph3.enter_context(tc.tile_pool(name="rd", bufs=2))
        rb_p = ph3.enter_context(tc.tile_pool(name="rb", bufs=2))
        ph2w = ExitStack()
        wv_p = ph2w.enter_context(tc.tile_pool(name="wvh", bufs=1))
        qkv_ps = ph2.enter_context(
            tc.tile_pool(name="qkv_ps", bufs=2, space="PSUM", side="right"))

        # ============ phase 1: LN1 (kv tiles first, then q) ============
        ph1 = ExitStack()
        PHASE_MARKS.append(("ph1", nc.next_id()))
        pools["stats"] = ph1.enter_context(tc.tile_pool(name="lnstats", bufs=4))
        tp_ps = ph1.enter_context(tc.tile_pool(name="tp_ps", bufs=2, space="PSUM"))
        xpool = ph1.enter_context(tc.tile_pool(name="lnx", bufs=3))
        zpool = ph1.enter_context(tc.tile_pool(name="lnz", bufs=3))

        def ln_kv_tile(tt, dstT, dstcol):
            xt = xpool.tile([128, EMB], F32, name="lnx")
            nc.sync.dma_start(out=xt, in_=d_xkv[tt * 128:(tt + 1) * 128, :])
            mu, rstd = _ln(nc, pools, xt, EMB, eps_t)
            zt = zpool.tile([128, EMB], BF, name="lnzt")
            nc.gpsimd.tensor_scalar(
                out=zt, in0=xt, scalar1=mu, scalar2=rstd, op0=SUB, op1=MUL)
            for ci in range(8):
                ps = tp_ps.tile([128, 128], BF, name="tp")
                nc.tensor.transpose(ps, zt[:, ci * 128:(ci + 1) * 128], ident)
                if ci % 2 == 0:
                    nc.scalar.copy(
                        out=dstT[:, ci, dstcol * 128:(dstcol + 1) * 128], in_=ps)
                else:
                    nc.vector.tensor_copy(
                        out=dstT[:, ci, dstcol * 128:(dstcol + 1) * 128], in_=ps)

        def ln_q_tile(tt):
            xt = xpool.tile([128, EMB], F32, name="lnx")
            nc.sync.dma_start(out=xt, in_=d_xq[tt * 128:(tt + 1) * 128, :])
            mu, rstd = _ln(nc, pools, xt, EMB, eps_t)
            zt = zpool.tile([128, EMB], F8, name="lnzt8")
            nc.gpsimd.tensor_scalar(
                out=zt, in0=xt, scalar1=mu, scalar2=rstd, op0=SUB, op1=MUL)
            for c in range(4):
                _u16t(nc, nc.sync,
                      zqT[:, c, tt * 128:(tt + 1) * 128, :],
                      zt[:, c * 256:(c + 1) * 256])

        for tt in range(NS):
            ln_kv_tile(tt, zkc[tt // 4], tt % 4)
        for tt in range(NJ):
            ln_q_tile(tt)
        ph1.close()
        st_ps = ph3.enter_context(tc.tile_pool(name="sT_ps", bufs=2, space="PSUM"))
        ot_psp = ph3.enter_context(tc.tile_pool(name="oT_ps", bufs=1, space="PSUM"))

        # ============ phase 2: V projection (overlaps LN tail) ============
        PHASE_MARKS.append(("ph2v", nc.next_id()))
        sc_v = SV / (S_WV * SZ)
        for oc in range(2):
            wv_sb = wv_p.tile([128, 8, 512], F8, name="wvh")
            nc.sync.dma_start(out=wv_sb, in_=d_wv[:, :, oc * 512:(oc + 1) * 512])
            for tt in range(NS):
                ps = qkv_ps.tile([128, 512], F32, name="vps", tag="qkvps")
                for c in range(4):
                    nc.tensor.matmul(
                        ps,
                        zkc[tt // 4][:, 2 * c:2 * c + 2,
                                     (tt % 4) * 128:(tt % 4 + 1) * 128],
                        wv_sb[:, 2 * c:2 * c + 2, :],
                        start=(c == 0), stop=(c == 3), perf_mode=DR)
                nc.vector.scalar_tensor_tensor(
                    out=v_sb[:, tt, oc * 8:(oc + 1) * 8, 0:64],
                    in0=ps.rearrange("p (h d) -> p h d", d=64), scalar=sc_v,
                    in1=bv_sb[:, oc * 512:(oc + 1) * 512]
                    .rearrange("p (h d) -> p h d", d=64),
                    op0=MUL, op1=ADD)
        ph2w.close()

        # ====== phase 3: per-4-head-group QK projection + attention ======
        PHASE_MARKS.append(("ph3", nc.next_id()))
        sc_q = SQ / (S_WQ * SZ)
        sc_k = SQ / (S_WK * SZ)

        for a in range(4):
            qt2 = qt_p.tile([128, 2, TQ], F8, name="qt2")
            kt2 = kt_p.tile([128, 2, T], F8, name="kt2")
            for half in range(2):
                ch = 2 * a + half
                wqt = wqk_p.tile([128, 4, 2, 128], F8, name="wqt", tag="wqk")
                nc.sync.dma_start(
                    out=wqt, in_=d_wq[:, :, :, ch * 128:(ch + 1) * 128])
                wkt = wqk_p.tile([128, 8, 128], F8, name="wkt", tag="wqk")
                nc.sync.dma_start(out=wkt, in_=d_wk[:, :, ch * 128:(ch + 1) * 128])
                for tc2 in range(2):
                    ps = qkv_ps.tile([128, 512], F32, name="qps", tag="qkvps")
                    for c in range(4):
                        nc.tensor.matmul(
                            ps, wqt[:, c, :, :],
                            zqT[:, c, tc2 * 512:(tc2 + 1) * 512, :]
                            .rearrange("p t two -> p two t"),
                            start=(c == 0), stop=(c == 3), perf_mode=DR)
                    nc.vector.tensor_scalar(
                        out=qt2[:, half, tc2 * 512:(tc2 + 1) * 512], in0=ps,
                        scalar1=sc_q, scalar2=bq_sb[:, ch:ch + 1],
                        op0=MUL, op1=ADD)
                for kc in range(4):
                    ps = qkv_ps.tile([128, 512], F32, name="kps", tag="qkvps")
                    for c in range(4):
                        nc.tensor.matmul(
                            ps, wkt[:, 2 * c:2 * c + 2, :],
                            zkc[kc][:, 2 * c:2 * c + 2, :],
                            start=(c == 0), stop=(c == 3), perf_mode=DR)
                    nc.vector.tensor_scalar(
                        out=kt2[:, half, kc * 512:(kc + 1) * 512], in0=ps,
                        scalar1=sc_k, scalar2=bk_sb[:, ch:ch + 1],
                        op0=MUL, op1=ADD)

            for j in range(4):
                h = 4 * a + j
                jb = 32 * j
                pt = pt_p.tile([128, PT_W], F8, name="pt")
                for s in range(NS):
                    m = s // 2
                    ps = st_ps.tile([128, 1024], F32, name="stps")
                    for (c0, c1) in _bank_spans(m):
                        nc.tensor.matmul(
                            ps[:, c0:c1],
                            kt2[jb:jb + 32, :, s * 128:(s + 1) * 128],
                            qt2[jb:jb + 32, :, c0:c1],
                            start=True, stop=True, perf_mode=DR,
                            tile_position=(jb, 0))
                    nc.scalar.activation(
                        out=pt[:, PT_OFF[s]:PT_OFF[s] + (NJ - m) * 128],
                        in_=ps[:, m * 128:1024], func=Exp,
                        bias=expb_t, scale=1.0 / (SQ * SQ))
                    nc.gpsimd.tensor_mul(
                        pt[:, PT_OFF[s]:PT_OFF[s] + 128],
                        pt[:, PT_OFF[s]:PT_OFF[s] + 128],
                        mm_sb[:, s, :])
                ot_ps = ot_psp.tile([96, TQ], F32, name="otps")
                for m in range(NJ):
                    w = (NJ - m) * 128
                    pp = pt[:, PT_OFF[2 * m]:PT_OFF[2 * m] + 2 * w].rearrange(
                        "p (two c) -> p two c", two=2)
                    for (c0, c1) in _bank_spans(m):
                        nc.tensor.matmul(
                            ot_ps[:, c0:c1],
                            v_sb[:, 2 * m:2 * m + 2, h, 0:96],
                            pp[:, :, c0 - m * 128:c1 - m * 128],
                            start=(m == 0), stop=(m == NJ - 1),
                            perf_mode=DR, skip_group_check=True)
                rd = rd_p.tile([1, TQ], F32, name="rd")
                nc.vector.reciprocal(out=rd, in_=ot_ps[64:65, :])
                rb = rb_p.tile([64, TQ], F32, name="rb")
                nc.gpsimd.partition_broadcast(rb, rd)
                nc.vector.tensor_mul(
                    oT_all[(h % 2) * 64:(h % 2) * 64 + 64, h // 2, :],
                    ot_ps[0:64, :], rb)
        ph2.close()
        ph3.close()
        stA.close()

        # ========= phase 4: Wo + residual + LN2 + z2 hi/lo =========
        PHASE_MARKS.append(("ph4", nc.next_id()))
        x2 = stX.enter_context(tc.tile_pool(name="x2", bufs=1, side="right")).tile(
            [128, 8, EMB], F32, name="x2_t")
        # z2T in u16-pair layout, hi/lo planes
        z2T = stX.enter_context(tc.tile_pool(name="z2T", bufs=1, side="right")).tile(
            [128, 2, 4, TQ, 2], F8, name="z2T_t")
        sc_o = 1.0 / (SO * S_WO)

        with ExitStack() as ph4:
            wo_p = ph4.enter_context(tc.tile_pool(name="wo", bufs=1))
            xq2_p = ph4.enter_context(tc.tile_pool(name="xq2", bufs=2))
            pools["stats"] = ph4.enter_context(
                tc.tile_pool(name="lnstats2", bufs=8))
            z2pool = ph4.enter_context(tc.tile_pool(name="lnz2", bufs=3))
            wo_ps = ph4.enter_context(
                tc.tile_pool(name="wo_ps", bufs=2, space="PSUM"))
            wo_sb = wo_p.tile([128, 2, 8, EMB], F8, name="wo_t")
            nc.sync.dma_start(out=wo_sb, in_=d_wo)

            for tt in range(NJ):
                xq_t = xq2_p.tile([128, EMB], F32, name="xq2")
                nc.sync.dma_start(out=xq_t, in_=d_xqb[tt * 128:(tt + 1) * 128, :])
                for cc in range(2):
                    ps = wo_ps.tile([128, 512], F32, name="wops")
                    nmm = 0
                    for pl in range(2):
                        for c in range(4):
                            nc.tensor.matmul(
                                ps, oT_all[:, 2 * c:2 * c + 2,
                                           tt * 128:(tt + 1) * 128],
                                wo_sb[:, pl, 2 * c:2 * c + 2,
                                      cc * 512:(cc + 1) * 512],
                                start=(nmm == 0), stop=(nmm == 7),
                                perf_mode=DR)
                            nmm += 1
                    nc.vector.scalar_tensor_tensor(
                        out=x2[:, tt, cc * 512:(cc + 1) * 512],
                        in0=ps, scalar=sc_o,
                        in1=xq_t[:, cc * 512:(cc + 1) * 512],
                        op0=MUL, op1=ADD)
                mu, rstd = _ln(nc, pools, x2[:, tt, :], EMB, eps_t)
                z2b = z2pool.tile([128, EMB], BF, name="z2b")
                nc.gpsimd.tensor_scalar(
                    out=z2b, in0=x2[:, tt, :], scalar1=mu, scalar2=rstd,
                    op0=SUB, op1=MUL)
                z2h = z2pool.tile([128, EMB], F8, name="z2h")
                nc.scalar.copy(out=z2h, in_=z2b)
                z2l = z2pool.tile([128, EMB], F8, name="z2l")
                nc.vector.tensor_sub(z2l, z2b, z2h)
                for si, zp in enumerate((z2h, z2l)):
                    for c in range(4):
                        _u16t(nc, nc.sync,
                              z2T[:, si, c, tt * 128:(tt + 1) * 128, :],
                              zp[:, c * 256:(c + 1) * 256])
        stO.close()
        stZ.close()
        stM.close()

        # ===== phase 5: MLP (3-term splits), u interleaved with first y =====
        PHASE_MARKS.append(("ph5a", nc.next_id()))
        uT = stX.enter_context(tc.tile_pool(name="uT", bufs=1, side="right")).tile(
            [128, 2, 32, TQ], F8, name="uT_t")
        zero_t = consts.tile([128, 1], F32)
        nc.vector.memset(zero_t, 0.0)
        sc_u = SU / (S_W1 * SZ)
        sc_y = 1.0 / (S_W2 * SU)
        with ExitStack() as ph5:
            w1_p = ph5.enter_context(tc.tile_pool(name="w1t", bufs=3))
            w2_p = ph5.enter_context(tc.tile_pool(name="w2h", bufs=1))
            ub_p = ph5.enter_context(tc.tile_pool(name="ub", bufs=3))
            u_ps = ph5.enter_context(
                tc.tile_pool(name="u_ps", bufs=4, space="PSUM", side="right"))
            y_ps = ph5.enter_context(
                tc.tile_pool(name="y_ps", bufs=4, space="PSUM", side="right"))
            yt_p = ph5.enter_context(tc.tile_pool(name="yt", bufs=4))

            def z2ap(si, c, tc2):
                return z2T[:, si, c, tc2 * 512:(tc2 + 1) * 512, :].rearrange(
                    "p t two -> p two t")

            def y_pass(w2h, cc, tts, with_u):
                pss = {}
                for tt in tts:
                    pss[tt] = y_ps.tile([128, 512], F32, name="ypst")
                for fp in range(16):
                    if with_u:
                        for fi in range(2):
                            ft = 2 * fp + fi
                            w1t = w1_p.tile([128, 2, 4, 2, 128], F8, name="w1t")
                            nc.sync.dma_start(out=w1t, in_=d_w1[ft])
                            for tc2 in range(2):
                                ps = u_ps.tile([128, 512], F32, name="upst")
                                nmm = 0
                                for c in range(4):  # hi*hi
                                    nc.tensor.matmul(
                                        ps, w1t[:, 0, c, :, :], z2ap(0, c, tc2),
                                        start=(nmm == 0), stop=False,
                                        perf_mode=DR)
                                    nmm += 1
                                for c in range(4):  # lo*hi + hi*lo
                                    nc.tensor.matmul(
                                        ps, w1t[:, 1, c, :, :], z2ap(0, c, tc2),
                                        start=False, stop=False, perf_mode=DR)
                                    nc.tensor.matmul(
                                        ps, w1t[:, 0, c, :, :], z2ap(1, c, tc2),
                                        start=False, stop=(c == 3),
                                        perf_mode=DR)
                                # u_hi fp8 + u_lo = relu - u_hi
                                sl = slice(tc2 * 512, (tc2 + 1) * 512)
                                nc.scalar.activation(
                                    out=uT[:, 0, ft, sl], in_=ps, func=Relu,
                                    bias=b1_sb[:, ft:ft + 1], scale=sc_u)
                                ub = ub_p.tile([128, 512], BF, name="ub")
                                nc.vector.tensor_scalar(
                                    out=ub, in0=ps, scalar1=sc_u, scalar2=0.0,
                                    op0=MUL, op1=MAX)
                                nc.gpsimd.tensor_sub(
                                    uT[:, 1, ft, sl], ub, uT[:, 0, ft, sl])
                    for tt in tts:
                        tsl = slice(tt * 128, (tt + 1) * 128)
                        nc.tensor.matmul(
                            pss[tt], uT[:, 0, 2 * fp:2 * fp + 2, tsl],
                            w2h[:, 0, 2 * fp:2 * fp + 2, :],
                            start=(fp == 0), stop=False, perf_mode=DR)
                        nc.tensor.matmul(
                            pss[tt], uT[:, 0, 2 * fp:2 * fp + 2, tsl],
                            w2h[:, 1, 2 * fp:2 * fp + 2, :],
                            start=False, stop=False, perf_mode=DR)
                        nc.tensor.matmul(
                            pss[tt], uT[:, 1, 2 * fp:2 * fp + 2, tsl],
                            w2h[:, 0, 2 * fp:2 * fp + 2, :],
                            start=False, stop=(fp == 15), perf_mode=DR)
                for tt in tts:
                    yt = yt_p.tile([128, 512], F32, name="yt")
                    nc.vector.scalar_tensor_tensor(
                        out=yt, in0=pss[tt], scalar=sc_y,
                        in1=x2[:, tt, cc * 512:(cc + 1) * 512],
                        op0=MUL, op1=ADD)
                    nc.sync.dma_start(
                        out=d_y[tt * 128:(tt + 1) * 128,
                                cc * 512:(cc + 1) * 512],
                        in_=yt)

            first = True
            for cc in range(2):
                w2h = w2_p.tile([128, 2, 32, 512], F8, name="w2h")
                nc.sync.dma_start(out=w2h, in_=d_w2[cc])
                for tq in range(2):
                    if not first:
                        PHASE_MARKS.append(("ph5b", nc.next_id()))
                    y_pass(w2h, cc, [4 * tq + i for i in range(4)],
                           with_u=first)
                    first = False

    nc.compile()
    return nc


_PROGRAM_CACHE = {}


def _get_program():
    if "nc" not in _PROGRAM_CACHE:
        _PROGRAM_CACHE["nc"] = build_program()
    return _PROGRAM_CACHE["nc"]


def _to_fp8(w, s, name):
    ws = np.asarray(w, np.float64) * s
    assert np.abs(ws).max() < 440.0, f"{name} fp8 overflow: {np.abs(ws).max()}"
    return ws.astype(np.float32).astype(FP8)


def _to_fp8_hilo(w, s, name):
    """[..., stacked hi/lo on a new leading axis]"""
    ws = (np.asarray(w, np.float64) * s).astype(np.float32)
    assert np.abs(ws).max() < 440.0, f"{name} fp8 overflow"
    hi = ws.astype(FP8)
    lo = (ws - hi.astype(np.float32)).astype(FP8)
    return np.stack([hi, lo], 0)


def _rowpair(w):  # [C, O] -> [128, 4, 2, O]  (e = 256c + 2p + i)
    O = w.shape[1]
    return np.ascontiguousarray(
        w.reshape(4, 128, 2, O).transpose(1, 0, 2, 3))


def _swz(w):  # [C, O] -> [128, 8, O]  (e = 128*ci + p)
    return np.ascontiguousarray(w.reshape(8, 128, -1).transpose(1, 0, 2))


def _host_prep(inputs):
    f32 = np.float32
    g1 = np.asarray(inputs["g1"], f32)
    be1 = np.asarray(inputs["be1"], f32)
    g2 = np.asarray(inputs["g2"], f32)
    be2 = np.asarray(inputs["be2"], f32)
    Wq = np.asarray(inputs["Wq"], f32)   # [H, C, HD]
    Wk = np.asarray(inputs["Wk"], f32)
    Wv = np.asarray(inputs["Wv"], f32).transpose(1, 0, 2).reshape(EMB, EMB)
    W1 = np.asarray(inputs["W1"], f32)
    W2 = np.asarray(inputs["W2"], f32)
    bo = np.asarray(inputs["bo"], f32)
    b2 = np.asarray(inputs["b2"], f32)
    rsc = np.sqrt(HD ** -0.5)

    def fold_qk(W):
        # [H, C, HD] -> [C, (a, half, j, d)] with H=4a+j, HD=32*half+d
        Wf = W.transpose(1, 0, 2).reshape(EMB, 4, 4, 2, 32)  # [C,a,j,half,d]
        return np.ascontiguousarray(
            Wf.transpose(0, 1, 3, 2, 4).reshape(EMB, EMB))

    Wq_f = fold_qk(Wq)
    Wk_f = fold_qk(Wk)
    w1_eff = g2[:, None] * W1
    w1_hilo = _to_fp8_hilo(w1_eff, S_W1, "w1")  # [2, C, FF]
    # -> [FF/128=32 ft, 2, 128, 4, 2, 128]
    w1_dev = np.ascontiguousarray(
        w1_hilo.reshape(2, 4, 128, 2, 32, 128)
        .transpose(4, 2, 0, 1, 3, 5))
    w2_hilo = _to_fp8_hilo(W2, S_W2, "w2")  # [2, FF, EMB]
    w2_dev = np.ascontiguousarray(
        w2_hilo.reshape(2, 32, 128, 2, 512).transpose(3, 2, 0, 1, 4))
    wo_hilo = _to_fp8_hilo(np.asarray(inputs["Wo"], f32), S_WO, "wo")
    wo_dev = np.stack([_swz(wo_hilo[0]), _swz(wo_hilo[1])], 1)

    com = {
        "wq": _rowpair(_to_fp8(g1[:, None] * Wq_f * rsc, S_WQ, "wq")),
        "wk": _swz(_to_fp8(g1[:, None] * Wk_f * rsc, S_WK, "wk")),
        "wv": _swz(_to_fp8(g1[:, None] * Wv, S_WV, "wv")),
        "wo": wo_dev,
        "w1": w1_dev,
        "w2": w2_dev,
        "bq": np.ascontiguousarray(
            ((be1 @ Wq_f) * rsc * SQ).reshape(8, 128).T.astype(f32)),
        "bk": np.ascontiguousarray(
            ((be1 @ Wk_f) * rsc * SQ).reshape(8, 128).T.astype(f32)),
        "b1s": np.ascontiguousarray(
            ((np.asarray(inputs["b1"], f32) + be2 @ W1) * SU)
            .reshape(32, 128).T.astype(f32)),
        "bvrow": ((be1 @ Wv) * SV).reshape(1, EMB).astype(f32),
    }

    masks = []
    for v in range(2):
        zig = ZIG[v]
        mm = np.zeros((NS, 128, 128), f32)
        tri = (np.arange(128)[:, None] <= np.arange(128)[None, :])
        for s in range(NS):
            g = zig[s // 2]
            if g > s:
                mm[s] = 1.0
            elif g == s:
                mm[s] = tri
        masks.append(np.ascontiguousarray(
            mm.transpose(1, 0, 2).astype(FP8)))

    x = np.asarray(inputs["x"], f32)
    in_maps = []
    for c in range(8):
        b, v = c // 2, c % 2
        zig = ZIG[v]
        x_kv = np.ascontiguousarray(x[b])
        x_q = np.ascontiguousarray(
            np.concatenate([x_kv[g * 128:(g + 1) * 128] for g in zig], 0))
        m = dict(com)
        m["x_q"] = x_q
        m["x_qb"] = x_q + bo[None, :]
        m["x_kv"] = x_kv
        m["maskm"] = masks[v]
        in_maps.append(m)
    return in_maps, b2


def kernel(**inputs) -> np.ndarray:
    nc = _get_program()
    in_maps, b2 = _host_prep(inputs)
    res = run_bass_kernel_spmd(nc, in_maps, core_ids=list(range(8)))
    out = np.zeros((B, T, EMB), np.float32)
    for c in range(8):
        b, v = c // 2, c % 2
        zig = ZIG[v]
        y = res.results[c]["y"]
        for j, g in enumerate(zig):
            out[b, g * 128:(g + 1) * 128] = y[j * 128:(j + 1) * 128]
    return out + b2[None, None, :]


# revision 42
# speedup vs baseline: 1.1395x; 1.1395x over previous
"""Trainium2 Bass kernel for a dense transformer block (B=4, T=2048, C=1024,
H=16, FF=4096, causal attention, fp32 I/O).

Sharding: data-parallel over 8 cores, 2 cores per batch, zigzag 128-row query
chunks (ZIG) to balance causal attention across the pair under one SPMD
program. K/V recomputed per core for the full batch.

Precision: fp8e4m3 DoubleRow matmuls everywhere except LN/softmax/residual
arithmetic. Error control:
- power-of-2 scales on every fp8 tensor, folded into existing epilogue
  scalars / the exp bias (zero extra ops);
- MLP runs 3-term split products (W_hi z_hi + W_lo z_hi + W_hi z_lo) so both
  weight- and activation-quantization errors cancel to O(fp8^2);
- attention keeps plain fp8 (softmax averaging suppresses the noise).
z^T layouts for Q and the MLP are produced by DMA-xbar transposes of fp8
pairs viewed as uint16, which lands tensors directly in DoubleRow pair
layout with no PE/ACT/DVE cost.
"""

import sys

for _p in ("/opt/trn_rl_repo",):
    if _p not in sys.path:
        sys.path.insert(0, _p)

import numpy as np
import ml_dtypes

import concourse.bass as bass
import concourse.mybir as mybir
import concourse.tile as tile
from concourse import bacc
from concourse.bass_utils import run_bass_kernel_spmd
from concourse.masks import make_identity

BF16 = ml_dtypes.bfloat16
FP8 = ml_dtypes.float8_e4m3fn
F32 = mybir.dt.float32
BF = mybir.dt.bfloat16
F8 = mybir.dt.float8e4
U16 = mybir.dt.uint16

EMB = 1024
HEADS = 16
HD = 64
FF = 4096
T = 2048
B = 4
EPS = 1e-5
TQ = 1024  # own query rows per core
NJ = 8  # own 128-row chunks per core
NS = 16  # key slots (128 keys each)
ZIG = [[0, 3, 4, 7, 8, 11, 12, 15], [1, 2, 5, 6, 9, 10, 13, 14]]

# physical fp8 scales (compile-time)
SZ = 8.0     # z1/z2
SQ = 4.0     # qt/kt
SP = 8.0     # p = exp
SV = 32.0    # v
SO = 32.0    # attention out (oT)
SU = 16.0    # u
EXP_BIAS = float(-5.0 + np.log(SP))  # exp(score/SQ^2 + EXP_BIAS)
S_WQ = 2048.0
S_WK = 2048.0
S_WV = 1024.0
S_WO = 1024.0
S_W1 = 1024.0
S_W2 = 2048.0

# packed P^T column offsets: slot s covers own-chunk range [s//2, 8)
PT_OFF = [0] * NS
for _s in range(1, NS):
    PT_OFF[_s] = PT_OFF[_s - 1] + (NJ - (_s - 1) // 2) * 128
PT_W = PT_OFF[-1] + (NJ - (NS - 1) // 2) * 128  # 9216

DR = mybir.MatmulPerfMode.DoubleRow


def _bank_spans(m):
    """fp32 PSUM bank-aligned column spans covering [m*128, 1024)."""
    if m < 4:
        return [(m * 128, 512), (512, 1024)]
    return [(m * 128, 1024)]


def _ln(nc, pools, xt, n_free, eps_t):
    """LayerNorm stats for xt [128, n_free] fp32 -> (mu, rstd*SZ) [128,1]."""
    stats = pools["stats"].tile([128, 2, 6], F32)
    half = n_free // 2
    nc.vector.bn_stats(out=stats[:, 0, :], in_=xt[:, 0:half])
    nc.vector.bn_stats(out=stats[:, 1, :], in_=xt[:, half:n_free])
    mv = pools["stats"].tile([128, 2], F32)
    nc.vector.bn_aggr(out=mv, in_=stats)
    rstd = pools["stats"].tile([128, 1], F32)
    nc.scalar.activation(
        out=rstd, in_=mv[:, 1:2], func=mybir.ActivationFunctionType.Sqrt,
        bias=eps_t, scale=1.0 / (SZ * SZ),
    )
    nc.vector.reciprocal(out=rstd, in_=rstd)
    return mv[:, 0:1], rstd


def _u16t(nc, eng, dst_ap, src_ap):
    """DMA-xbar transpose of an fp8 tile viewed as uint16 pairs.
    src [128, 2W] fp8 row-major -> dst [128, W, 2] fp8: dst[p, t, i] =
    src[t, 2p+i]."""
    eng.dma_start_transpose(
        out=dst_ap.rearrange("p a b -> p (a b)").bitcast(U16),
        in_=src_ap.bitcast(U16))


PHASE_MARKS = []


def build_program():
    from contextlib import ExitStack

    nc = bacc.Bacc("TRN2", target_bir_lowering=False, debug=False, num_devices=1)

    d_xq = nc.dram_tensor("x_q", [TQ, EMB], BF, kind="ExternalInput").ap()
    d_xqb = nc.dram_tensor("x_qb", [TQ, EMB], F32, kind="ExternalInput").ap()
    d_xkv = nc.dram_tensor("x_kv", [T, EMB], BF, kind="ExternalInput").ap()
    # weights host-swizzled (fp8 + scales); wq/w1 in row-pair layouts
    d_wq = nc.dram_tensor("wq", [128, 4, 2, EMB], F8, kind="ExternalInput").ap()
    d_wk = nc.dram_tensor("wk", [128, 8, EMB], F8, kind="ExternalInput").ap()
    d_wv = nc.dram_tensor("wv", [128, 8, EMB], F8, kind="ExternalInput").ap()
    d_wo = nc.dram_tensor("wo", [128, 2, 8, EMB], F8, kind="ExternalInput").ap()
    d_w1 = nc.dram_tensor(
        "w1", [32, 128, 2, 8, 128], F8, kind="ExternalInput").ap()
    d_w2 = nc.dram_tensor(
        "w2", [2, 128, 2, 32, 512], F8, kind="ExternalInput").ap()
    d_bq = nc.dram_tensor("bq", [128, 8], F32, kind="ExternalInput").ap()
    d_bk = nc.dram_tensor("bk", [128, 8], F32, kind="ExternalInput").ap()
    d_b1 = nc.dram_tensor("b1s", [128, 32], F32, kind="ExternalInput").ap()
    d_bv = nc.dram_tensor("bvrow", [1, EMB], F32, kind="ExternalInput").ap()
    d_mm = nc.dram_tensor("maskm", [128, NS, 128], F8, kind="ExternalInput").ap()
    d_y = nc.dram_tensor("y", [TQ, EMB], F32, kind="ExternalOutput").ap()

    Exp = mybir.ActivationFunctionType.Exp
    Relu = mybir.ActivationFunctionType.Relu
    MUL = mybir.AluOpType.mult
    ADD = mybir.AluOpType.add
    SUB = mybir.AluOpType.subtract
    MAX = mybir.AluOpType.max

    with tile.TileContext(nc) as tc, ExitStack() as top:
        consts = top.enter_context(tc.tile_pool(name="consts", bufs=1))
        ident = consts.tile([128, 128], BF)
        make_identity(nc, ident)
        eps_t = consts.tile([128, 1], F32)
        nc.vector.memset(eps_t, EPS)
        expb_t = consts.tile([128, 1], F32)
        nc.vector.memset(expb_t, EXP_BIAS)
        bq_sb = consts.tile([128, 8], F32)
        nc.sync.dma_start(out=bq_sb, in_=d_bq)
        bk_sb = consts.tile([128, 8], F32)
        nc.sync.dma_start(out=bk_sb, in_=d_bk)
        b1_sb = consts.tile([128, 32], F32)
        nc.sync.dma_start(out=b1_sb, in_=d_b1)
        stM = ExitStack()
        mm_sb = stM.enter_context(tc.tile_pool(name="maskp", bufs=1)).tile(
            [128, NS, 128], F8, name="mm_sb")
        nc.sync.dma_start(out=mm_sb, in_=d_mm)

        def bcast_row(dst, src_row):
            b_ap = bass.AP(
                tensor=src_row.tensor, offset=src_row.offset,
                ap=[[0, 128]] + list(src_row.ap[1:]))
            nc.gpsimd.dma_start(out=dst, in_=b_ap)

        bv_sb = consts.tile([128, EMB], F32)
        bcast_row(bv_sb, d_bv)

        pools = {}

        stZ = ExitStack()   # z^T tensors + v: die after attention
        stO = ExitStack()   # oT_all: dies after Wo
        stX = ExitStack()   # x2/z2T/uT: die at end
        top.enter_context(stX)
        top.enter_context(stO)
        top.enter_context(stZ)

        oT_all = stO.enter_context(tc.tile_pool(name="oT", bufs=1)).tile(
            [128, 8, TQ], F8, name="oT_t")
        wo_pre = stO.enter_context(tc.tile_pool(name="wo", bufs=1))
        wo_sb = wo_pre.tile([128, 2, 8, EMB], F8, name="wo_t")

        # zq in u16-transposed pair layout [128, chunk, tok, 2]
        zq_p = stZ.enter_context(tc.tile_pool(name="zqT", bufs=1))
        zkv_p = stZ.enter_context(tc.tile_pool(name="zkvT", bufs=1))
        zqT = zq_p.tile([128, 4, TQ, 2], F8, name="zqT")
        zkc = [zkv_p.tile([128, 8, 512], F8, name=f"zkc{i}") for i in range(4)]

        v_sb = stZ.enter_context(
            tc.tile_pool(name="v", bufs=1)).tile(
            [128, NS, HEADS, 96], F8, name="v_t")
        nc.vector.memset(v_sb[:, :, :, 64:65], SV / SO)

        wv_p = stZ.enter_context(tc.tile_pool(name="wvh", bufs=1))
        wv_sbs = []
        for oc in range(2):
            wv_sb = wv_p.tile([128, 8, 512], F8, name="wvh")
            nc.scalar.dma_start(
                out=wv_sb, in_=d_wv[:, :, oc * 512:(oc + 1) * 512])
            wv_sbs.append(wv_sb)

        ph2 = ExitStack()
        ph3 = ExitStack()
        wqk_p = ph3.enter_context(tc.tile_pool(name="wqk", bufs=2))
        qt_p = ph3.enter_context(tc.tile_pool(name="qTot", bufs=2))
        kt_p = ph3.enter_context(tc.tile_pool(name="kTot", bufs=2))
        pt_p = ph3.enter_context(tc.tile_pool(name="pT", bufs=2))
        rd_p = ph3.enter_context(tc.tile_pool(name="rd", bufs=2))
        rb_p = ph3.enter_context(tc.tile_pool(name="rb", bufs=2))
        qkv_ps = ph2.enter_context(
            tc.tile_pool(name="qkv_ps", bufs=2, space="PSUM", side="right"))

        # ============ phase 1: LN1 (kv tiles first, then q) ============
        ph1 = ExitStack()
        PHASE_MARKS.append(("ph1", nc.next_id()))
        pools["stats"] = ph1.enter_context(tc.tile_pool(name="lnstats", bufs=4))
        tp_ps = ph1.enter_context(tc.tile_pool(name="tp_ps", bufs=2, space="PSUM"))
        xpool = ph1.enter_context(tc.tile_pool(name="lnx", bufs=3))
        zpool = ph1.enter_context(tc.tile_pool(name="lnz", bufs=3))

        def ln_kv_tile(xt, tt, dstT, dstcol):
            mu, rstd = _ln(nc, pools, xt, EMB, eps_t)
            zt = zpool.tile([128, EMB], BF, name="lnzt")
            nc.gpsimd.tensor_scalar(
                out=zt, in0=xt, scalar1=mu, scalar2=rstd, op0=SUB, op1=MUL)
            for ci in range(8):
                ps = tp_ps.tile([128, 128], BF, name="tp")
                nc.tensor.transpose(ps, zt[:, ci * 128:(ci + 1) * 128], ident)
                nc.scalar.copy(
                    out=dstT[:, ci, dstcol * 128:(dstcol + 1) * 128], in_=ps)

        def ln_q_tile(xt, tt):
            mu, rstd = _ln(nc, pools, xt, EMB, eps_t)
            zt = zpool.tile([128, EMB], F8, name="lnzt8")
            nc.gpsimd.tensor_scalar(
                out=zt, in0=xt, scalar1=mu, scalar2=rstd, op0=SUB, op1=MUL)
            for c in range(4):
                _u16t(nc, nc.scalar,
                      zqT[:, c, tt * 128:(tt + 1) * 128, :],
                      zt[:, c * 256:(c + 1) * 256])

        def kv_group(g, split=False):
            if split:  # fast first tile
                xc = xpool.tile([128, 4, EMB], BF, name="lnx")
                nc.sync.dma_start(
                    out=xc[:, 0, :], in_=d_xkv[0:128, :])
                nc.sync.dma_start(
                    out=xc[:, 1:4, :], in_=d_xkv[128:512, :]
                    .rearrange("(t p) c -> p t c", p=128))
            else:
                xc = xpool.tile([128, 4, EMB], BF, name="lnx")
                nc.sync.dma_start(
                    out=xc, in_=d_xkv[g * 512:(g + 1) * 512, :]
                    .rearrange("(t p) c -> p t c", p=128))
            for i in range(4):
                ln_kv_tile(xc[:, i, :], 4 * g + i, zkc[g], i)

        def q_group(g):
            xc = xpool.tile([128, 4, EMB], BF, name="lnx")
            nc.sync.dma_start(
                out=xc, in_=d_xq[g * 512:(g + 1) * 512, :]
                .rearrange("(t p) c -> p t c", p=128))
            for i in range(4):
                ln_q_tile(xc[:, i, :], 4 * g + i)

        kv_group(0, split=True)
        kv_group(1)
        kv_group(2)
        kv_group(3)
        q_group(0)
        q_group(1)
        ph1.close()
        st_ps = ph3.enter_context(tc.tile_pool(name="sT_ps", bufs=2, space="PSUM"))
        ot_psp = ph3.enter_context(tc.tile_pool(name="oT_ps", bufs=1, space="PSUM"))

        # ============ phase 2: V projection (overlaps LN tail) ============
        PHASE_MARKS.append(("ph2v", nc.next_id()))
        sc_v = SV / (S_WV * SZ)
        for oc in range(2):
            wv_sb = wv_sbs[oc]
            for tt in range(NS):
                ps = qkv_ps.tile([128, 512], F32, name="vps", tag="qkvps")
                for c in range(4):
                    nc.tensor.matmul(
                        ps,
                        zkc[tt // 4][:, 2 * c:2 * c + 2,
                                     (tt % 4) * 128:(tt % 4 + 1) * 128],
                        wv_sb[:, 2 * c:2 * c + 2, :],
                        start=(c == 0), stop=(c == 3), perf_mode=DR)
                nc.vector.scalar_tensor_tensor(
                    out=v_sb[:, tt, oc * 8:(oc + 1) * 8, 0:64],
                    in0=ps.rearrange("p (h d) -> p h d", d=64), scalar=sc_v,
                    in1=bv_sb[:, oc * 512:(oc + 1) * 512]
                    .rearrange("p (h d) -> p h d", d=64),
                    op0=MUL, op1=ADD)

        # ====== phase 3: per-4-head-group QK projection + attention ======
        PHASE_MARKS.append(("ph3", nc.next_id()))
        sc_q = SQ / (S_WQ * SZ)
        sc_k = SQ / (S_WK * SZ)

        # prefetch MLP weights during attention (DMA bus is idle here)
        w15_p = stX.enter_context(tc.tile_pool(name="w1t", bufs=1, side="right"))
        w25_p = stX.enter_context(tc.tile_pool(name="w2h", bufs=2, side="right"))
        w2hs = [w25_p.tile([128, 2, 32, 512], F8, name="w2h")
                for cc in range(2)]
        w1_tiles = []

        def w1_fetch(g):
            w1t = w15_p.tile([128, 4, 2, 8, 128], F8, name="w1t")
            nc.gpsimd.dma_start(
                out=w1t, in_=d_w1[4 * g:4 * g + 4]
                .rearrange("f p a b c -> p f a b c"))
            w1_tiles.append(w1t)

        def stagger_prefetch(a):
            if a == 0:
                nc.gpsimd.dma_start(out=wo_sb, in_=d_wo)
            elif a == 1:
                nc.gpsimd.dma_start(out=w2hs[0], in_=d_w2[0])
            elif a == 2:
                nc.gpsimd.dma_start(out=w2hs[1], in_=d_w2[1])
            elif a == 3:
                w1_fetch(0)
                w1_fetch(1)

        for a in range(4):
            stagger_prefetch(a)
            qt2 = qt_p.tile([128, 2, TQ], F8, name="qt2")
            kt2 = kt_p.tile([128, 2, T], F8, name="kt2")
            for half in range(2):
                ch = 2 * a + half
                wqt = wqk_p.tile([128, 4, 2, 128], F8, name="wqt", tag="wqk")
                nc.sync.dma_start(
                    out=wqt, in_=d_wq[:, :, :, ch * 128:(ch + 1) * 128])
                wkt = wqk_p.tile([128, 8, 128], F8, name="wkt", tag="wqk")
                nc.sync.dma_start(out=wkt, in_=d_wk[:, :, ch * 128:(ch + 1) * 128])
                for tc2 in range(2):
                    ps = qkv_ps.tile([128, 512], F32, name="qps", tag="qkvps")
                    for c in range(4):
                        nc.tensor.matmul(
                            ps, wqt[:, c, :, :],
                            zqT[:, c, tc2 * 512:(tc2 + 1) * 512, :]
                            .rearrange("p t two -> p two t"),
                            start=(c == 0), stop=(c == 3), perf_mode=DR)
                    nc.vector.tensor_scalar(
                        out=qt2[:, half, tc2 * 512:(tc2 + 1) * 512], in0=ps,
                        scalar1=sc_q, scalar2=bq_sb[:, ch:ch + 1],
                        op0=MUL, op1=ADD)
                for kc in range(4):
                    ps = qkv_ps.tile([128, 512], F32, name="kps", tag="qkvps")
                    for c in range(4):
                        nc.tensor.matmul(
                            ps, wkt[:, 2 * c:2 * c + 2, :],
                            zkc[kc][:, 2 * c:2 * c + 2, :],
                            start=(c == 0), stop=(c == 3), perf_mode=DR)
                    nc.vector.tensor_scalar(
                        out=kt2[:, half, kc * 512:(kc + 1) * 512], in0=ps,
                        scalar1=sc_k, scalar2=bk_sb[:, ch:ch + 1],
                        op0=MUL, op1=ADD)

            for j in range(4):
                h = 4 * a + j
                jb = 32 * j
                pt = pt_p.tile([128, PT_W], F8, name="pt")
                for s in range(NS):
                    m = s // 2
                    ps = st_ps.tile([128, 1024], F32, name="stps")
                    for (c0, c1) in _bank_spans(m):
                        nc.tensor.matmul(
                            ps[:, c0:c1],
                            kt2[jb:jb + 32, :, s * 128:(s + 1) * 128],
                            qt2[jb:jb + 32, :, c0:c1],
                            start=True, stop=True, perf_mode=DR,
                            tile_position=(jb, 0))
                    nc.scalar.activation(
                        out=pt[:, PT_OFF[s]:PT_OFF[s] + (NJ - m) * 128],
                        in_=ps[:, m * 128:1024], func=Exp,
                        bias=expb_t, scale=1.0 / (SQ * SQ))
                    nc.gpsimd.tensor_mul(
                        pt[:, PT_OFF[s]:PT_OFF[s] + 128],
                        pt[:, PT_OFF[s]:PT_OFF[s] + 128],
                        mm_sb[:, s, :])
                ot_ps = ot_psp.tile([96, TQ], F32, name="otps")
                for m in range(NJ):
                    w = (NJ - m) * 128
                    pp = pt[:, PT_OFF[2 * m]:PT_OFF[2 * m] + 2 * w].rearrange(
                        "p (two c) -> p two c", two=2)
                    for (c0, c1) in _bank_spans(m):
                        nc.tensor.matmul(
                            ot_ps[:, c0:c1],
                            v_sb[:, 2 * m:2 * m + 2, h, 0:96],
                            pp[:, :, c0 - m * 128:c1 - m * 128],
                            start=(m == 0), stop=(m == NJ - 1),
                            perf_mode=DR, skip_group_check=True)
                rd = rd_p.tile([1, TQ], F32, name="rd")
                nc.vector.reciprocal(out=rd, in_=ot_ps[64:65, :])
                rb = rb_p.tile([64, TQ], F32, name="rb")
                nc.gpsimd.partition_broadcast(rb, rd)
                nc.vector.tensor_mul(
                    oT_all[(h % 2) * 64:(h % 2) * 64 + 64, h // 2, :],
                    ot_ps[0:64, :], rb)
        ph2.close()
        ph3.close()
        stZ.close()

        # ========= phase 4: Wo + residual + LN2 + z2 hi/lo =========
        PHASE_MARKS.append(("ph4", nc.next_id()))
        x2 = stX.enter_context(tc.tile_pool(name="x2", bufs=1, side="right")).tile(
            [128, 8, EMB], F32, name="x2_t")
        z2T = stX.enter_context(tc.tile_pool(name="z2T", bufs=1, side="right")).tile(
            [128, 2, 8, TQ], F8, name="z2T_t")
        sc_o = 1.0 / (SO * S_WO)

        with ExitStack() as ph4:
            xq2_p = ph4.enter_context(tc.tile_pool(name="xq2", bufs=1))
            pools["stats"] = ph4.enter_context(
                tc.tile_pool(name="lnstats2", bufs=8))
            z2pool = ph4.enter_context(tc.tile_pool(name="lnz2", bufs=2))
            wo_ps = ph4.enter_context(
                tc.tile_pool(name="wo_ps", bufs=2, space="PSUM"))
            tp2_ps = ph4.enter_context(
                tc.tile_pool(name="tp2_ps", bufs=2, space="PSUM"))
            for tg in range(2):
                xq_t4 = xq2_p.tile([128, 4, EMB], F32, name="xq2")
                nc.sync.dma_start(
                    out=xq_t4, in_=d_xqb[tg * 512:(tg + 1) * 512, :]
                    .rearrange("(t p) c -> p t c", p=128))
                for ti in range(4):
                    tt = 4 * tg + ti
                    xq_t = xq_t4[:, ti, :]
                    for cc in range(2):
                        ps = wo_ps.tile([128, 512], F32, name="wops")
                        nmm = 0
                        for pl in range(2):
                            for c in range(4):
                                nc.tensor.matmul(
                                    ps, oT_all[:, 2 * c:2 * c + 2,
                                               tt * 128:(tt + 1) * 128],
                                    wo_sb[:, pl, 2 * c:2 * c + 2,
                                          cc * 512:(cc + 1) * 512],
                                    start=(nmm == 0), stop=(nmm == 7),
                                    perf_mode=DR)
                                nmm += 1
                        nc.vector.scalar_tensor_tensor(
                            out=x2[:, tt, cc * 512:(cc + 1) * 512],
                            in0=ps, scalar=sc_o,
                            in1=xq_t[:, cc * 512:(cc + 1) * 512],
                            op0=MUL, op1=ADD)
                    mu, rstd = _ln(nc, pools, x2[:, tt, :], EMB, eps_t)
                    z2b = z2pool.tile([128, EMB], BF, name="z2b")
                    nc.gpsimd.tensor_scalar(
                        out=z2b, in0=x2[:, tt, :], scalar1=mu, scalar2=rstd,
                        op0=SUB, op1=MUL)
                    z2h = z2pool.tile([128, EMB], F8, name="z2h")
                    nc.scalar.copy(out=z2h, in_=z2b)
                    z2l = z2pool.tile([128, EMB], BF, name="z2l")
                    nc.gpsimd.tensor_sub(z2l, z2b, z2h)
                    for si, zp in enumerate((z2b, z2l)):
                        for ci in range(8):
                            ps = tp2_ps.tile([128, 128], BF, name="tp2")
                            nc.tensor.transpose(
                                ps, zp[:, ci * 128:(ci + 1) * 128], ident)
                            if ci % 4 != 3:
                                nc.scalar.copy(
                                    out=z2T[:, si, ci,
                                            tt * 128:(tt + 1) * 128],
                                    in_=ps)
                            else:
                                nc.vector.tensor_copy(
                                    out=z2T[:, si, ci,
                                            tt * 128:(tt + 1) * 128],
                                    in_=ps)
        stO.close()
        stM.close()

        # ===== phase 5: MLP (3-term splits), u interleaved with first y =====
        PHASE_MARKS.append(("ph5a", nc.next_id()))
        uT = stX.enter_context(tc.tile_pool(name="uT", bufs=1, side="right")).tile(
            [128, 2, 32, TQ], F8, name="uT_t")
        zero_t = consts.tile([128, 1], F32)
        nc.vector.memset(zero_t, 0.0)
        sc_u = SU / (S_W1 * SZ)
        sc_y = 1.0 / (S_W2 * SU)
        with ExitStack() as ph5:
            ub_p = ph5.enter_context(tc.tile_pool(name="ub", bufs=2))
            u_ps = ph5.enter_context(
                tc.tile_pool(name="u_ps", bufs=4, space="PSUM", side="right"))
            y_ps = ph5.enter_context(
                tc.tile_pool(name="y_ps", bufs=4, space="PSUM", side="right"))
            yt_p = ph5.enter_context(tc.tile_pool(name="yt", bufs=2))

            def z2ap(si, c, tc2):
                return z2T[:, si, 2 * c:2 * c + 2,
                           tc2 * 512:(tc2 + 1) * 512]

            def y_pass(w2h, cc, tts, with_u):
                pss = {}
                for tt in tts:
                    pss[tt] = y_ps.tile([128, 512], F32, name="ypst")
                for fp in range(16):
                    if with_u:
                        if fp % 2 == 0 and fp // 2 + 2 < 8:
                            w1_fetch(fp // 2 + 2)
                        for fi in range(2):
                            ft = 2 * fp + fi
                            w1t = w1_tiles[ft // 4][:, ft % 4]  # [128,2,8,128]
                            for tc2 in range(2):
                                ps = u_ps.tile([128, 512], F32, name="upst")
                                nmm = 0
                                for c in range(4):  # hi*hi
                                    nc.tensor.matmul(
                                        ps, w1t[:, 0, 2 * c:2 * c + 2, :],
                                        z2ap(0, c, tc2),
                                        start=(nmm == 0), stop=False,
                                        perf_mode=DR)
                                    nmm += 1
                                for c in range(4):  # lo*hi + hi*lo
                                    nc.tensor.matmul(
                                        ps, w1t[:, 1, 2 * c:2 * c + 2, :],
                                        z2ap(0, c, tc2),
                                        start=False, stop=False, perf_mode=DR)
                                    nc.tensor.matmul(
                                        ps, w1t[:, 0, 2 * c:2 * c + 2, :],
                                        z2ap(1, c, tc2),
                                        start=False, stop=(c == 3),
                                        perf_mode=DR)
                                # u_hi fp8 + u_lo = relu - u_hi
                                sl = slice(tc2 * 512, (tc2 + 1) * 512)
                                nc.scalar.activation(
                                    out=uT[:, 0, ft, sl], in_=ps, func=Relu,
                                    bias=b1_sb[:, ft:ft + 1], scale=sc_u)
                                ub = ub_p.tile([128, 512], BF, name="ub")
                                nc.vector.tensor_scalar(
                                    out=ub, in0=ps, scalar1=sc_u, scalar2=0.0,
                                    op0=MUL, op1=MAX)
                                nc.gpsimd.tensor_sub(
                                    uT[:, 1, ft, sl], ub, uT[:, 0, ft, sl])
                    for tt in tts:
                        tsl = slice(tt * 128, (tt + 1) * 128)
                        nc.tensor.matmul(
                            pss[tt], uT[:, 0, 2 * fp:2 * fp + 2, tsl],
                            w2h[:, 0, 2 * fp:2 * fp + 2, :],
                            start=(fp == 0), stop=False, perf_mode=DR)
                        nc.tensor.matmul(
                            pss[tt], uT[:, 0, 2 * fp:2 * fp + 2, tsl],
                            w2h[:, 1, 2 * fp:2 * fp + 2, :],
                            start=False, stop=False, perf_mode=DR)
                        nc.tensor.matmul(
                            pss[tt], uT[:, 1, 2 * fp:2 * fp + 2, tsl],
                            w2h[:, 0, 2 * fp:2 * fp + 2, :],
                            start=False, stop=(fp == 15), perf_mode=DR)
                for tt in tts:
                    yt = yt_p.tile([128, 512], F32, name="yt")
                    nc.vector.scalar_tensor_tensor(
                        out=yt, in0=pss[tt], scalar=sc_y,
                        in1=x2[:, tt, cc * 512:(cc + 1) * 512],
                        op0=MUL, op1=ADD)
                    nc.sync.dma_start(
                        out=d_y[tt * 128:(tt + 1) * 128,
                                cc * 512:(cc + 1) * 512],
                        in_=yt)

            first = True
            for cc in range(2):
                for tq in range(2):
                    if not first:
                        PHASE_MARKS.append(("ph5b", nc.next_id()))
                    y_pass(w2hs[cc], cc, [4 * tq + i for i in range(4)],
                           with_u=first)
                    first = False

    nc.compile()
    return nc


_PROGRAM_CACHE = {}


def _get_program():
    if "nc" not in _PROGRAM_CACHE:
        _PROGRAM_CACHE["nc"] = build_program()
    return _PROGRAM_CACHE["nc"]


def _to_fp8(w, s, name):
    ws = np.asarray(w, np.float64) * s
    assert np.abs(ws).max() < 440.0, f"{name} fp8 overflow: {np.abs(ws).max()}"
    return ws.astype(np.float32).astype(FP8)


def _to_fp8_hilo(w, s, name):
    """[..., stacked hi/lo on a new leading axis]"""
    ws = (np.asarray(w, np.float64) * s).astype(np.float32)
    assert np.abs(ws).max() < 440.0, f"{name} fp8 overflow"
    hi = ws.astype(FP8)
    lo = (ws - hi.astype(np.float32)).astype(FP8)
    return np.stack([hi, lo], 0)


def _rowpair(w):  # [C, O] -> [128, 4, 2, O]  (e = 256c + 2p + i)
    O = w.shape[1]
    return np.ascontiguousarray(
        w.reshape(4, 128, 2, O).transpose(1, 0, 2, 3))


def _swz(w):  # [C, O] -> [128, 8, O]  (e = 128*ci + p)
    return np.ascontiguousarray(w.reshape(8, 128, -1).transpose(1, 0, 2))


def _host_prep(inputs):
    f32 = np.float32
    g1 = np.asarray(inputs["g1"], f32)
    be1 = np.asarray(inputs["be1"], f32)
    g2 = np.asarray(inputs["g2"], f32)
    be2 = np.asarray(inputs["be2"], f32)
    Wq = np.asarray(inputs["Wq"], f32)   # [H, C, HD]
    Wk = np.asarray(inputs["Wk"], f32)
    Wv = np.asarray(inputs["Wv"], f32).transpose(1, 0, 2).reshape(EMB, EMB)
    W1 = np.asarray(inputs["W1"], f32)
    W2 = np.asarray(inputs["W2"], f32)
    bo = np.asarray(inputs["bo"], f32)
    b2 = np.asarray(inputs["b2"], f32)
    rsc = np.sqrt(HD ** -0.5)

    def fold_qk(W):
        # [H, C, HD] -> [C, (a, half, j, d)] with H=4a+j, HD=32*half+d
        Wf = W.transpose(1, 0, 2).reshape(EMB, 4, 4, 2, 32)  # [C,a,j,half,d]
        return np.ascontiguousarray(
            Wf.transpose(0, 1, 3, 2, 4).reshape(EMB, EMB))

    Wq_f = fold_qk(Wq)
    Wk_f = fold_qk(Wk)
    w1_eff = g2[:, None] * W1
    w1_hilo = _to_fp8_hilo(w1_eff, S_W1, "w1")  # [2, C, FF]
    # -> [FF/128=32 ft, 2, 128, 4, 2, 128]
    w1_dev = np.ascontiguousarray(
        w1_hilo.reshape(2, 8, 128, 32, 128).transpose(3, 2, 0, 1, 4))
    w2_hilo = _to_fp8_hilo(W2, S_W2, "w2")  # [2, FF, EMB]
    w2_dev = np.ascontiguousarray(
        w2_hilo.reshape(2, 32, 128, 2, 512).transpose(3, 2, 0, 1, 4))
    wo_hilo = _to_fp8_hilo(np.asarray(inputs["Wo"], f32), S_WO, "wo")
    wo_dev = np.stack([_swz(wo_hilo[0]), _swz(wo_hilo[1])], 1)

    com = {
        "wq": _rowpair(_to_fp8(g1[:, None] * Wq_f * rsc, S_WQ, "wq")),
        "wk": _swz(_to_fp8(g1[:, None] * Wk_f * rsc, S_WK, "wk")),
        "wv": _swz(_to_fp8(g1[:, None] * Wv, S_WV, "wv")),
        "wo": wo_dev,
        "w1": w1_dev,
        "w2": w2_dev,
        "bq": np.ascontiguousarray(
            ((be1 @ Wq_f) * rsc * SQ).reshape(8, 128).T.astype(f32)),
        "bk": np.ascontiguousarray(
            ((be1 @ Wk_f) * rsc * SQ).reshape(8, 128).T.astype(f32)),
        "b1s": np.ascontiguousarray(
            ((np.asarray(inputs["b1"], f32) + be2 @ W1) * SU)
            .reshape(32, 128).T.astype(f32)),
        "bvrow": ((be1 @ Wv) * SV).reshape(1, EMB).astype(f32),
    }

    masks = []
    for v in range(2):
        zig = ZIG[v]
        mm = np.zeros((NS, 128, 128), f32)
        tri = (np.arange(128)[:, None] <= np.arange(128)[None, :])
        for s in range(NS):
            g = zig[s // 2]
            if g > s:
                mm[s] = 1.0
            elif g == s:
                mm[s] = tri
        masks.append(np.ascontiguousarray(
            mm.transpose(1, 0, 2).astype(FP8)))

    x = np.asarray(inputs["x"], f32)
    in_maps = []
    for c in range(8):
        b, v = c // 2, c % 2
        zig = ZIG[v]
        x_kv = np.ascontiguousarray(x[b])
        x_q = np.ascontiguousarray(
            np.concatenate([x_kv[g * 128:(g + 1) * 128] for g in zig], 0))
        m = dict(com)
        m["x_q"] = x_q.astype(BF16)
        m["x_qb"] = x_q + bo[None, :]
        m["x_kv"] = x_kv.astype(BF16)
        m["maskm"] = masks[v]
        in_maps.append(m)
    return in_maps, b2


def kernel(**inputs) -> np.ndarray:
    nc = _get_program()
    in_maps, b2 = _host_prep(inputs)
    res = run_bass_kernel_spmd(nc, in_maps, core_ids=list(range(8)))
    out = np.zeros((B, T, EMB), np.float32)
    for c in range(8):
        b, v = c // 2, c % 2
        zig = ZIG[v]
        y = res.results[c]["y"]
        for j, g in enumerate(zig):
            out[b, g * 128:(g + 1) * 128] = y[j * 128:(j + 1) * 128]
    return out + b2[None, None, :]
